# revision 1
# baseline (speedup 1.0000x reference)
"""Game-of-Life CNN (3x3 circular conv + double-heaviside) on 8 trn2 cores.

Multi-path hybrid, one path per engine group, split by image rows:

  BIT-path (DVE only): 16 image columns packed per u16 word; partition
    axis = 128 column-groups, free axis = image rows, so vertical
    neighbor shifts are free AP offsets.  Host supplies the grid plus
    left/right column-rotated copies (layout-only work), and the cell
    update is a 26-op bitwise full-adder network:
        rowsum (3:2): s0/s1;  colsum of three 2-bit rowsums -> S0,S1,S2
        alive = (S0&S1&~S2) | (x & ~S0&~S1&S2)   [S==3 | (S==4 & x)]
    Output is bit-packed u16 (0.125 B/px each way).

  F8-path (PE + ACT/POOL): u8 {0,1} pixels reinterpreted as f8e4m3
    denormals (0x01 = 2^-9).  The full 3x3 conv is THREE accumulating
    matmuls with banded lhsT (stencil column weights along the
    partition axis; the column shift comes from a shifted rhs view),
    PSUM = count * 2^-9 exactly, split into two [128, 1024] PSUM halves
    for pipeline depth 4.  Threshold per half:
      ACT:   q = Square(256*p - 3) = (count/2 - 3)^2   (exact in bf16)
      then   out = u8(Relu(1.3 - q)) on ACT, or u8(q <= 0.3) on POOL
    both give {0,1} exactly under truncation or rounding.

Row split per image: rows [0, HB) -> BIT, [HB, 2048) -> F8; f8 tiles
alternate the second window op between ACT and POOL; output DMAs are
issued from the POOL queue so they never stall the SP load queue.
All engines (DVE, PE, ACT, POOL, DMA) run concurrently.
"""

import numpy as np
import ml_dtypes

import bass_rust
import concourse.bass as bass
import concourse.bacc as bacc
import concourse.mybir as mybir
from concourse import tile
from concourse.alu_op_type import AluOpType as A
from concourse.bass_utils import run_bass_kernel_spmd

B, H, W = 16, 2048, 2048
NCORES = 8
IPC = B // NCORES          # images per core
U16 = mybir.dt.uint16
U8 = mybir.dt.uint8
F8E4 = mybir.dt.float8e4
F32 = mybir.dt.float32
BF16 = mybir.dt.bfloat16
AF = mybir.ActivationFunctionType

# --- tuning knobs ---------------------------------------------------------
TROWS = 126                # f8 output rows per tile
HF = 504                   # f8-path rows per image (PE + ACT/POOL)
NT_F8 = (HF + TROWS - 1) // TROWS
HB = H - HF                # bit-path rows per image (DVE)
# per-tile window engine: 'A' (ACT) or 'P' (POOL), cycled
F8_PATTERN = ['A', 'P']

SEG = HB + 2               # bit-plane columns per image (rows + halo)
DDUP = 2062                # second x copy offset; DR halves stride 2064


def _build_nc():
    nc = bacc.Bacc()
    pl = nc.dram_tensor("pl", [128, IPC * SEG], U16, kind="ExternalInput")
    p0 = nc.dram_tensor("p0", [128, IPC * SEG], U16, kind="ExternalInput")
    pr = nc.dram_tensor("pr", [128, IPC * SEG], U16, kind="ExternalInput")
    outb = nc.dram_tensor("outb", [128, IPC * HB], U16, kind="ExternalOutput")
    x8 = nc.dram_tensor("x8", [IPC * (HF + 2), DDUP], U8, kind="ExternalInput")
    wmat = nc.dram_tensor("wmat", [128, 3 * 128], F8E4, kind="ExternalInput")
    y8 = nc.dram_tensor("y8", [IPC * HF, W], U8, kind="ExternalOutput")

    FW = IPC * SEG          # full free width of bit planes

    with tile.TileContext(nc) as tc:
        with (
            tc.tile_pool(name="const", bufs=1) as cpool,
            tc.tile_pool(name="bp", bufs=1) as bpool,     # bit planes + temps
            tc.tile_pool(name="x8p", bufs=3) as xpool,
            tc.tile_pool(name="qp", bufs=3) as qpool,
            tc.tile_pool(name="op", bufs=4) as opool,
            tc.tile_pool(name="ps", bufs=2, space="PSUM") as pspool,
        ):
            # ---- bit path: load planes first (DVE critical path) ----
            A_ = bpool.tile([128, FW], U16, tag="A")
            B_ = bpool.tile([128, FW], U16, tag="B")
            C_ = bpool.tile([128, FW], U16, tag="C")
            # per-image loads, A/B of image 0 first: the first two DVE
            # ops run on image 0's half while image 1 and C still load
            nc.sync.dma_start(out=A_[:, 0:SEG], in_=pl[:, 0:SEG])
            nc.sync.dma_start(out=B_[:, 0:SEG], in_=p0[:, 0:SEG])
            nc.sync.dma_start(out=A_[:, SEG:FW], in_=pl[:, SEG:FW])
            nc.sync.dma_start(out=B_[:, SEG:FW], in_=p0[:, SEG:FW])
            nc.sync.dma_start(out=C_[:, :], in_=pr[:, :])

            # ---- constants ----
            wsb = cpool.tile([128, 3 * 128], F8E4, tag="w")
            nc.sync.dma_start(out=wsb[:, :], in_=wmat[:, :])
            bias_q = cpool.tile([128, 1], F32, tag="bq")
            nc.vector.memset(bias_q[:, :], -3.0)
            bias_r = cpool.tile([128, 1], F32, tag="br")
            nc.vector.memset(bias_r[:, :], 1.3)

            # ---- f8 path (interleave issue; engines run concurrently) ----
            def f8_tiles():
                for img in range(IPC):
                    for t in range(NT_F8):
                        r0 = t * TROWS
                        n_out = min(TROWS, HF - r0)
                        yield img, r0, n_out, F8_PATTERN[t % len(F8_PATTERN)]

            for img, r0, n_out, eng in f8_tiles():
                n_in = n_out + 2
                xt = xpool.tile([128, W + 2], U8, tag="x")
                rlo, rhi = img * (HF + 2) + r0, img * (HF + 2) + r0 + n_in
                nc.sync.dma_start(out=xt[0:n_in, :], in_=x8[rlo:rhi, 0:W + 2])
                xf = xt[:, :].bitcast(F8E4)
                o = opool.tile([128, W], U8, tag="o")
                # full-width psum; plain 3-pass conv per 512-chunk,
                # all dst at partition base 0; full-width windows
                ps = pspool.tile([128, W], F32, tag="ps",
                                 name=f"ps_{img}_{r0}")
                for ch in range(4):
                    c0 = ch * 512
                    for dc in range(3):
                        nc.tensor.matmul(
                            ps[0:n_out, c0:c0 + 512],
                            lhsT=wsb[0:n_in, dc * 128:dc * 128 + n_out],
                            rhs=xf[0:n_in, dc + c0: dc + c0 + 512],
                            start=(dc == 0), stop=(dc == 2),
                            skip_group_check=True)
                q = qpool.tile([128, W], BF16, tag="q",
                               name=f"q_{img}_{r0}")
                nc.scalar.activation(q[0:n_out, :], ps[0:n_out, :],
                                     AF.Square, bias=bias_q[0:n_out, :],
                                     scale=256.0)
                if eng == 'A':
                    nc.scalar.activation(o[0:n_out, :], q[0:n_out, :],
                                         AF.Relu, bias=bias_r[0:n_out, :],
                                         scale=-1.0)
                else:
                    nc.gpsimd.tensor_scalar(o[0:n_out, :], q[0:n_out, :],
                                            0.3, None, A.is_le, A.bypass)
                nc.gpsimd.dma_start(
                    out=y8[img * HF + r0: img * HF + r0 + n_out, :],
                    in_=o[0:n_out, :])

            # ---- bit path: 26-op DVE network over merged planes ----
            # 9 physical buffers (A/B/C planes + T1..T6), reused by
            # lifetime; all [128, FW], stage>=2 tensors use [:, 0:M].
            xor_, and_, or_ = A.bitwise_xor, A.bitwise_and, A.bitwise_or
            V = nc.vector
            M = FW - 2      # interior width

            _n = [0]

            def buf(tag):
                _n[0] += 1
                return bpool.tile([128, FW], U16, tag=tag,
                                  name=f"bb{_n[0]}_{tag}")

            t_ = buf("T1")
            V.tensor_tensor(t_[:, 0:SEG], A_[:, 0:SEG], B_[:, 0:SEG], xor_)
            u_ = buf("T3")
            V.tensor_tensor(u_[:, 0:SEG], A_[:, 0:SEG], B_[:, 0:SEG], and_)
            V.tensor_tensor(t_[:, SEG:FW], A_[:, SEG:FW], B_[:, SEG:FW], xor_)
            V.tensor_tensor(u_[:, SEG:FW], A_[:, SEG:FW], B_[:, SEG:FW], and_)
            v_ = buf("T4")
            V.tensor_tensor(v_[:, :], t_[:, :], C_[:, :], and_)
            s0 = buf("T2")
            V.tensor_tensor(s0[:, :], t_[:, :], C_[:, :], xor_)
            s1 = buf("T5")
            V.tensor_tensor(s1[:, :], u_[:, :], v_[:, :], or_)

            s0u, s0c, s0d = s0[:, 0:M], s0[:, 1:M + 1], s0[:, 2:M + 2]
            s1u, s1c, s1d = s1[:, 0:M], s1[:, 1:M + 1], s1[:, 2:M + 2]

            t1 = buf("T1")          # t dead
            V.tensor_tensor(t1[:, 0:M], s0u, s0d, xor_)
            m1 = buf("T4")          # v dead
            V.tensor_tensor(m1[:, 0:M], s0u, s0d, and_)
            S0 = buf("T3")          # u dead
            V.tensor_tensor(S0[:, 0:M], t1[:, 0:M], s0c, xor_)
            m2 = buf("A")           # A plane dead
            V.tensor_tensor(m2[:, 0:M], t1[:, 0:M], s0c, and_)
            t2 = buf("T6")
            V.tensor_tensor(t2[:, 0:M], s1u, s1d, xor_)
            c0 = buf("C")           # C plane dead
            V.tensor_tensor(c0[:, 0:M], m1[:, 0:M], m2[:, 0:M], or_)
            m3 = buf("T4")          # m1 dead
            V.tensor_tensor(m3[:, 0:M], s1u, s1d, and_)
            x1 = buf("T1")          # t1 dead
            V.tensor_tensor(x1[:, 0:M], t2[:, 0:M], s1c, xor_)
            m4 = buf("A")           # m2 dead
            V.tensor_tensor(m4[:, 0:M], t2[:, 0:M], s1c, and_)
            S1 = buf("T2")          # s0 dead
            V.tensor_tensor(S1[:, 0:M], x1[:, 0:M], c0[:, 0:M], xor_)
            c1a = buf("T5")         # s1 dead
            V.tensor_tensor(c1a[:, 0:M], m3[:, 0:M], m4[:, 0:M], or_)
            c1b = buf("T6")         # t2 dead
            V.tensor_tensor(c1b[:, 0:M], x1[:, 0:M], c0[:, 0:M], and_)
            # stage 3: S in {3,4} <=> ~(S0^S1) & (S1^S2); then & (S0|x)
            S2 = buf("T4")          # m3 dead
            V.tensor_tensor(S2[:, 0:M], c1a[:, 0:M], c1b[:, 0:M], xor_)
            D_ = buf("T1")          # x1 dead
            V.tensor_tensor(D_[:, 0:M], S0[:, 0:M], S1[:, 0:M], xor_)
            G2 = buf("A")           # m4 dead
            V.tensor_tensor(G2[:, 0:M], S2[:, 0:M], S1[:, 0:M], xor_)
            nD = buf("C")           # c0 dead
            V.tensor_scalar(nD[:, 0:M], D_[:, 0:M], 65535, None, xor_, A.bypass)
            sx = buf("T5")          # c1a dead
            V.tensor_tensor(sx[:, 0:M], S0[:, 0:M], B_[:, 1:M + 1], or_)
            c1_ = buf("T2")         # S1 dead
            V.tensor_tensor(c1_[:, 0:M], G2[:, 0:M], nD[:, 0:M], and_)
            # final AND split per image so each image's output DMA
            # overlaps the other half's compute
            alive = buf("T6")       # c1b dead
            V.tensor_tensor(alive[:, 0:SEG], c1_[:, 0:SEG], sx[:, 0:SEG], and_)
            nc.gpsimd.dma_start(out=outb[:, 0:HB], in_=alive[:, 0:HB])
            V.tensor_tensor(alive[:, SEG:M], c1_[:, SEG:M], sx[:, SEG:M], and_)
            nc.gpsimd.dma_start(out=outb[:, HB:2 * HB],
                                in_=alive[:, SEG:SEG + HB])

            # alive[:, j] = row j+1 of merged planes; img k interior at
            # merged cols [k*SEG+1, k*SEG+1+HB) -> alive cols [k*SEG, ...)
    nc.finalize()
    return nc


def _weight_mats(wk: np.ndarray) -> np.ndarray:
    """[128, 384] f8e4: three banded lhsT (stencil columns L, C, R).

    lhsT[k, i] = weight of input tile row k for output row i (stencil
    row k-i in 0..2); pass dc reads rhs shifted by dc so column dc pairs
    with weight column dc.
    """
    mats = []
    for dc in range(3):
        m = np.zeros((128, 128), np.float32)
        for k in range(128):
            for i in range(max(0, k - 2), min(k + 1, 126)):
                m[k, i] = wk[k - i, dc]
        mats.append(m)
    return np.concatenate(mats, axis=1).astype(ml_dtypes.float8_e4m3)
def _pack_bits(plane: np.ndarray) -> np.ndarray:
    """[rows, 2048] {0,1} -> [128, rows] u16 (16 cols per word)."""
    r = plane.shape[0]
    v = plane.reshape(r, 128, 16).astype(np.uint16)
    w = (v << np.arange(16, dtype=np.uint16)).sum(axis=2, dtype=np.uint16)
    return np.ascontiguousarray(w.T)


def _host_pack(xc: np.ndarray):
    """xc: [IPC, H, W] uint8 -> input arrays for one core."""
    pls, p0s, prs, x8s = [], [], [], []
    rows_b = np.arange(-1, HB + 1) % H           # bit-path rows + halo
    rows_f = np.arange(HB - 1, HB + HF + 1) % H  # f8-path rows + halo
    cols_f = np.arange(-1, W + 1) % W
    for k in range(IPC):
        img = xc[k]
        p0s.append(_pack_bits(img[rows_b]))
        pls.append(_pack_bits(np.roll(img, 1, axis=1)[rows_b]))
        prs.append(_pack_bits(np.roll(img, -1, axis=1)[rows_b]))
        xp = np.zeros((HF + 2, DDUP), np.uint8)
        xp[:, 0:W + 2] = img[np.ix_(rows_f, cols_f)]
        x8s.append(xp)
    return (np.concatenate(pls, axis=1), np.concatenate(p0s, axis=1),
            np.concatenate(prs, axis=1), np.concatenate(x8s, axis=0))


def _host_unpack(outb: np.ndarray, y8: np.ndarray) -> np.ndarray:
    """Device outputs -> [IPC, H, W] float32 for one core."""
    out = np.empty((IPC, H, W), np.float32)
    for k in range(IPC):
        w = outb[:, k * HB:(k + 1) * HB].T       # [HB, 128]
        bits = (w[:, :, None] >> np.arange(16, dtype=np.uint16)) & 1
        out[k, :HB] = bits.reshape(HB, W)
        out[k, HB:] = y8[k * HF:(k + 1) * HF]
    return out


def _run(inputs, trace=False, **kw):
    x = np.asarray(inputs["x"]).reshape(B, H, W)
    wk = np.asarray(
        inputs.get("kernel",
                   np.array([[2., 2., 2.], [2., 1., 2.], [2., 2., 2.]]))
    ).reshape(3, 3).astype(np.float32)
    # bias only shifts the thresholds by <1/3; count is integer so the
    # alive set {5,6,7} is unchanged for any |bias| < 0.5 (checked below)
    bias = float(np.asarray(inputs.get("bias", np.zeros(1))).reshape(-1)[0])
    assert abs(bias) < 0.5

    nc = _build_nc()
    wmat = _weight_mats(wk)
    xb = (x != 0).astype(np.uint8)
    in_maps = []
    for c in range(NCORES):
        pl, p0, pr, x8 = _host_pack(xb[c * IPC:(c + 1) * IPC])
        in_maps.append({"pl": pl, "p0": p0, "pr": pr, "x8": x8, "wmat": wmat})
    res = run_bass_kernel_spmd(nc, in_maps, core_ids=list(range(NCORES)),
                               trace=trace, **kw)
    out = np.empty((B, 1, H, W), np.float32)
    for c in range(NCORES):
        out[c * IPC:(c + 1) * IPC, 0] = _host_unpack(
            res.results[c]["outb"], res.results[c]["y8"])
    return out, res


def kernel(**inputs) -> np.ndarray:
    out, _ = _run(inputs, trace=False)
    return out

